# revision 25
# baseline (speedup 1.0000x reference)
"""Trainium2 8-core MoE layer kernel (collapsed shared-expert FFN, Bass/Tile).

The reference MoE applies the SAME w1/b1/w2/b2 to every expert's slice of
the dispatch buffer, so the whole layer collapses algebraically to

    out[t] = s(t) * (gelu(x[t] @ w1.T + b1) @ w2.T + b2)
    s(t)   = p0(t)*keep0(t) + p1(t)*keep1(t)

where keep_k(t) is the capacity-survival flag from the slot-major
cumulative-count over the global top-2 assignment sequence.  Only s(t)
needs global routing info; the FFN itself is a dense per-token FFN.

Sharding: token-parallel, 256 tokens per core.  Each core runs the
fp32 router on its own tokens, AllGathers the 4KB of routing decisions,
recomputes the global cumsum locally (replicated, deterministic), and
runs the dense FFN on its 256 tokens with w1/w2 streamed from HBM in
bf16.  The routing/scan chain runs on vector/scalar/gpsimd engines
concurrently with the F1 matmul stream; s(t) is only consumed by the
final epilogue scaling.
"""
import numpy as np
import ml_dtypes

import concourse.bass as bass
import concourse.mybir as mybir
import concourse.tile as tile
import concourse.bass_isa as bass_isa

_PATCH_DOC = """Patch TileContext._drain_and_barrier: the stock version stuffs every
outstanding semaphore wait onto one SP Drain instruction; the installed
walrus rejects >1 sync wait per non-EventSemaphore instruction
("Too many sync wait commands"). Split the waits across a chain of SP
nops, then drain/barrier as before."""
import concourse.tile as tile_mod
from concourse.vector_clock import ScopedClock


def _patched_drain_and_barrier(self, tick_clock, wait_clock):
    nc = self.nc
    carrier = nc.sync.nop(nofuse=True, hint="drain_wait_carrier")
    wait_clock.add_sem_waits(
        carrier.ins, ScopedClock({None: tick_clock.global_clock})
    )
    waits = list(carrier.ins.sync_info.on_wait)
    if len(waits) > 1:
        carrier.ins.sync_info.on_wait = waits[:1]
        import bass_rust as _br
        for w in waits[1:]:
            extra = nc.sync.nop(nofuse=True, hint="drain_wait_carrier")
            extra.ins.sync_info = _br.SyncInfo(on_wait=[w], on_update=[])

    nc.sync.drain()
    nc.all_engine_barrier()
    assert self.sems is not None
    popped = nc._tile_sem_poison_stack.pop()
    assert popped is self._sem_poison
    nc.clear_and_free_semaphores(list(self.sems.allocated().values()))
    nc.all_engine_barrier()


def apply():
    tile_mod.TileContext._drain_and_barrier = _patched_drain_and_barrier


import concourse.mybir as mybir
import bass_rust as _br


def split_multi_waits(nc):
    """Walrus in this container accepts at most ONE sync wait per
    instruction. Hoist extra waits onto same-engine NoOps inserted
    immediately before the offending instruction."""
    ctr = 0
    for f in nc.m.functions:
        for b in f.blocks:
            insts = b.instructions
            need = any(
                inst.sync_info is not None and len(inst.sync_info.on_wait) > 1
                for inst in insts
            )
            if not need:
                continue
            out = []
            for inst in insts:
                si = inst.sync_info
                if si is not None and len(si.on_wait) > 1:
                    waits = list(si.on_wait)
                    for w in waits[:-1]:
                        nop = mybir.InstNoOp(name=f"I-wsplit-{ctr}", ins=[], outs=[])
                        ctr += 1
                        nop.engine = inst.engine
                        nop.sync_info = _br.SyncInfo(on_wait=[w], on_update=[])
                        out.append(nop)
                    si.on_wait = waits[-1:]
                out.append(inst)
            b.instructions = out
    return ctr


E, TOPK, CAP, H, F, N, NCORES = 8, 2, 512, 2048, 8192, 2048, 8
TT = N // 128                # 16 token tiles (global)
HT = H // 128                # 16 hidden tiles
FT = F // 128                # 64 ffn tiles
TOKC = N // NCORES           # 256 tokens per core
MT = TOKC // 128             # 2 local token tiles

f32 = mybir.dt.float32
f16 = mybir.dt.float16
bf16 = mybir.dt.bfloat16
i32 = mybir.dt.int32
AOP = mybir.AluOpType
AFT = mybir.ActivationFunctionType
AX = mybir.AxisListType


def build_moe(nc: bass.Bass):
    xtm = nc.dram_tensor("xtm", [MT, 128, H], f32, kind="ExternalInput")
    xtc = nc.dram_tensor("xtc", [128, HT * TOKC], bf16, kind="ExternalInput")
    rwT = nc.dram_tensor("rwT", [H, E], f32, kind="ExternalInput")
    w1T = nc.dram_tensor("w1tt", [FT, 128, HT * 128], bf16, kind="ExternalInput")
    w2T = nc.dram_tensor("w2T", [F, H], bf16, kind="ExternalInput")
    b1t = nc.dram_tensor("b1t", [128, FT], f32, kind="ExternalInput")
    b2r = nc.dram_tensor("b2r", [1, H], f32, kind="ExternalInput")
    cid = nc.dram_tensor("cid", [1, 1], f32, kind="ExternalInput")
    out = nc.dram_tensor("out", [TOKC, H], f32, kind="ExternalOutput")

    rloc = nc.dram_tensor("rloc", [2, TOKC], f32)
    rall = nc.dram_tensor("rall", [2 * NCORES, TOKC], f32, addr_space="Shared")
    posd = nc.dram_tensor("posd", [1, 2 * N], f32)
    posq = nc.dram_tensor("posq", [2 * MT, 128], f32)

    with tile.TileContext(nc, num_cores=NCORES) as tc:
        with tc.tile_pool(name="persist", bufs=1) as persist:
            _body(nc, tc, persist, xtm, xtc, rwT, w1T, w2T, b1t, b2r, cid,
                  out, rloc, rall, posd, posq)
    return nc


def _body(nc, tc, persist, xtm, xtc, rwT, w1T, w2T, b1t, b2r, cid, out,
          rloc, rall, posd, posq):
    # ---- persistent tiles ----
    xts = persist.tile([128, HT * TOKC], bf16, tag="xts")      # xT own tokens
    b2b = persist.tile([128, H], f32, tag="b2b")
    b2s = [persist.tile([128, H], f32, tag=f"b2s_{m}", name=f"b2s_{m}")
           for m in range(MT)]
    cidb4 = persist.tile([2 * MT, 1], f32, tag="cidb4")
    b1sb = persist.tile([128, FT], f32, tag="b1sb")
    rws = persist.tile([128, HT * E], f32, tag="rws")
    sm = [persist.tile([128, 1], f32, tag=f"sm_{m}", name=f"sm_{m}")
          for m in range(MT)]
    prb = [persist.tile([128, 2], f32, tag=f"prb_{m}", name=f"prb_{m}")
           for m in range(MT)]
    lsb2 = [persist.tile([128, E], f32, tag=f"lsb2_{m}", name=f"lsb2_{m}")
            for m in range(MT)]
    nm2 = [persist.tile([128, 1], f32, tag=f"nm2_{m}", name=f"nm2_{m}")
           for m in range(MT)]
    posti = persist.tile([128, 2 * MT], f32, tag="posti")

    # pools in LIFO lifetime order (longest-lived entered first)
    g_cm = tc.tile_pool(name="g", bufs=1)
    g_pool = g_cm.__enter__()
    w2_cm = tc.tile_pool(name="w2s", bufs=6)
    w2_pool = w2_cm.__enter__()
    scan_cm = tc.tile_pool(name="scan", bufs=1)
    scan_pool = scan_cm.__enter__()
    w1_cm = tc.tile_pool(name="w1s", bufs=10)
    w1_pool = w1_cm.__enter__()
    f1ps_cm = tc.tile_pool(name="f1ps", bufs=6, space="PSUM")
    f1ps_pool = f1ps_cm.__enter__()

    # initial small loads: gpsimd handles router-critical, scalar the rest
    nc.gpsimd.dma_start(out=cidb4[:],
                        in_=cid[0:1, :].partition_broadcast(2 * MT).opt())
    nc.gpsimd.dma_start(out=rws[:].rearrange("p (c e) -> p c e", e=E),
                        in_=rwT[:, :].rearrange("(c p) e -> p c e", p=128))
    nc.scalar.dma_start(out=b1sb[:], in_=b1t[:, :])
    HH = HT * TOKC // 2
    nc.gpsimd.dma_start(out=xts[:, :HH], in_=xtc[:, :HH])
    nc.gpsimd.dma_start(out=xts[:, HH:], in_=xtc[:, HH:])

    w1t = {}

    def load_w1(ft):
        t = w1_pool.tile([128, HT * 128], bf16, tag="w1t")
        w1t[ft] = t
        eng = nc.sync if (ft % 2 == 0) else nc.scalar
        eng.dma_start(out=t[:], in_=w1T[ft, :, :])

    for ft in range(8):
        load_w1(ft)

    g = [g_pool.tile([128, TOKC], bf16, tag=f"g_{ft}", name=f"g_{ft}")
         for ft in range(FT)]

    def f1_block(ft):
        if ft + 8 < FT:
            load_w1(ft + 8)
        ps = f1ps_pool.tile([128, TOKC], f32, tag="f1ps")
        for hc in range(HT):
            nc.tensor.matmul(
                out=ps[:], lhsT=w1t[ft][:, hc * 128:(hc + 1) * 128],
                rhs=xts[:, hc * TOKC:(hc + 1) * TOKC],
                start=(hc == 0), stop=(hc == HT - 1))
        nc.scalar.activation(out=g[ft][:], in_=ps[:], func=AFT.Gelu,
                             bias=b1sb[:, ft:ft + 1], scale=1.0)

    # F1 head: two tiles before the router matmuls so the PE queue never
    # waits on the (slightly later) xtm DMA.
    f1_block(0)
    f1_block(1)

    # ============ Router (own 256 tokens, fp32) + AllGather ============
    # Only the top-2 INDICES gate the AllGather (softmax is monotonic);
    # probabilities are computed after F1, overlapped with F2.
    with (tc.tile_pool(name="r_x", bufs=2) as r_x,
          tc.tile_pool(name="r_ps", bufs=2, space="PSUM") as r_ps,
          tc.tile_pool(name="r_sb", bufs=2) as r_sb):
        xtt = []
        for tt2 in range(MT):
            xt_t = r_x.tile([128, H], f32, tag="xt_t", name=f"xt_t{tt2}")
            nc.gpsimd.dma_start(out=xt_t[:], in_=xtm[tt2, :, :])
            xtt.append(xt_t)
        for tt2 in range(MT):
            ps = r_ps.tile([128, E], f32, tag="r_ps")
            for hc in range(HT):
                nc.tensor.matmul(
                    out=ps[:], lhsT=xtt[tt2][:, hc * 128:(hc + 1) * 128],
                    rhs=rws[:, hc * E:(hc + 1) * E],
                    start=(hc == 0), stop=(hc == HT - 1))
            nc.vector.tensor_copy(out=lsb2[tt2][:], in_=ps[:])
            mx = r_sb.tile([128, 1], f32, tag="mx")
            nc.vector.tensor_reduce(out=mx[:], in_=lsb2[tt2][:], op=AOP.max,
                                    axis=AX.X)
            nc.vector.tensor_scalar_mul(nm2[tt2][:], mx[:], -1.0)
            mx8 = r_sb.tile([128, 8], f32, tag="mx8")
            ix8 = r_sb.tile([128, 8], mybir.dt.uint32, tag="ix8")
            nc.vector.max_with_indices(out_max=mx8[:], out_indices=ix8[:],
                                       in_=lsb2[tt2][:])
            ev = r_sb.tile([128, 2], f32, tag="ev")
            nc.vector.tensor_copy(out=ev[:, 0:1], in_=ix8[:, 0:1])
            nc.vector.tensor_copy(out=ev[:, 1:2], in_=ix8[:, 1:2])
            nc.gpsimd.dma_start(
                out=rloc[:, tt2 * 128:(tt2 + 1) * 128].rearrange("a t -> t a"),
                in_=ev[:])
        nc.gpsimd.collective_compute(
            "AllGather", AOP.bypass,
            replica_groups=[list(range(NCORES))],
            ins=[rloc[:, :].opt()],
            outs=[rall[:, :].opt()])

    # w2 prefetch for the F1->F2 boundary (gpsimd queue, right after AG)
    w2t = {}

    def load_w2(fc, eng=None):
        t = w2_pool.tile([128, H], bf16, tag="w2t")
        w2t[fc] = t
        if eng is None:
            eng = nc.sync if (fc % 2 == 0) else nc.scalar
        eng.dma_start(out=t[:], in_=w2T[fc * 128:(fc + 1) * 128, :])

    for fc in range(4):
        load_w2(fc, eng=nc.gpsimd)

    # ============ Scan: global slot-major position per assignment ======
    # (vector engine; runs concurrently with F1 matmuls)
    iop8 = scan_pool.tile([E, 1], i32, tag="iop8")
    iop8f = scan_pool.tile([E, 1], f32, tag="iop8f")
    nc.gpsimd.iota(iop8[:], pattern=[[0, 1]], base=0, channel_multiplier=1)
    nc.vector.tensor_copy(out=iop8f[:], in_=iop8[:])
    e0b = scan_pool.tile([E, N], f16, tag="e0b")
    e1b = scan_pool.tile([E, N], f16, tag="e1b")
    rallv = rall[:, :].rearrange("(c a) t -> a c t", a=2)
    nc.gpsimd.dma_start(
        out=e0b[:].rearrange("p (c t) -> p c t", t=TOKC),
        in_=rallv[0:1, :, :].partition_broadcast(E).opt())
    nc.gpsimd.dma_start(
        out=e1b[:].rearrange("p (c t) -> p c t", t=TOKC),
        in_=rallv[1:2, :, :].partition_broadcast(E).opt())
    ohcat = scan_pool.tile([E, 2 * N], f16, tag="ohcat")
    nc.vector.tensor_scalar(out=ohcat[:, :N], in0=e0b[:], scalar1=iop8f[:],
                            scalar2=None, op0=AOP.is_equal)
    nc.vector.tensor_scalar(out=ohcat[:, N:], in0=e1b[:], scalar1=iop8f[:],
                            scalar2=None, op0=AOP.is_equal)
    ones2n = scan_pool.tile([E, 2 * N], f16, tag="ones2n")
    nc.vector.memset(ones2n[:], 1.0)
    cum = scan_pool.tile([E, 2 * N], f16, tag="cum")
    nc.vector.tensor_tensor_scan(out=cum[:], data0=ones2n[:], data1=ohcat[:],
                                 initial=0.0, op0=AOP.mult, op1=AOP.add)
    ohcum = scan_pool.tile([E, 2 * N], f16, tag="ohcum")
    nc.vector.tensor_tensor(out=ohcum[:], in0=ohcat[:], in1=cum[:],
                            op=AOP.mult)
    ones8 = scan_pool.tile([E, 1], f16, tag="ones8")
    nc.vector.memset(ones8[:], 1.0)
    posrow = scan_pool.tile([1, 2 * N], f32, tag="posrow")

    # ============ F1 body (tiles 2..63) ============
    for ft in range(2, FT):
        f1_block(ft)

    f1ps_cm.__exit__(None, None, None)
    w1_cm.__exit__(None, None, None)

    # pos extraction matmuls after the F1 stream (the AllGather + scan
    # complete well before F1 ends, so these never stall the PE).
    with tc.tile_pool(name="csps", bufs=4, space="PSUM") as csps:
        for ch in range(2 * N // 512):
            pps = csps.tile([1, 512], f32, tag="pps")
            nc.tensor.matmul(out=pps[:], lhsT=ones8[:],
                             rhs=ohcum[:, ch * 512:(ch + 1) * 512],
                             start=True, stop=True)
            nc.vector.tensor_scalar_add(
                posrow[:, ch * 512:(ch + 1) * 512], pps[:], -1.0)
    nc.gpsimd.dma_start(out=posd[:, :], in_=posrow[:])

    # ============ own-token keep/s: gather pos rows {2c+m, 16+2c+m} ======
    with tc.tile_pool(name="imath", bufs=1) as im:
        # late softmax: probabilities for own tokens (overlaps F2)
        for m in range(MT):
            ex = im.tile([128, E], f32, tag="ex", name=f"ex_{m}")
            ssum = im.tile([128, 1], f32, tag="ssum", name=f"ssum_{m}")
            nc.scalar.activation(out=ex[:], in_=lsb2[m][:], func=AFT.Exp,
                                 bias=nm2[m][:], scale=1.0, accum_out=ssum[:])
            rcp = im.tile([128, 1], f32, tag="rcp", name=f"rcp_{m}")
            nc.vector.reciprocal(out=rcp[:], in_=ssum[:])
            pr = im.tile([128, E], f32, tag="pr", name=f"pr_{m}")
            nc.vector.tensor_scalar_mul(pr[:], ex[:], rcp[:])
            mx8 = im.tile([128, 8], f32, tag="mx8", name=f"mx8_{m}")
            ix8d = im.tile([128, 8], mybir.dt.uint32, tag="ix8d",
                           name=f"ix8d_{m}")
            nc.vector.max_with_indices(out_max=mx8[:], out_indices=ix8d[:],
                                       in_=pr[:])
            nc.vector.tensor_copy(out=prb[m][:], in_=mx8[:, 0:2])

        iop4 = im.tile([2 * MT, 1], i32, tag="iop4")
        iop4f = im.tile([2 * MT, 1], f32, tag="iop4f")
        nc.gpsimd.iota(iop4[:], pattern=[[0, 1]], base=0, channel_multiplier=1)
        nc.vector.tensor_copy(out=iop4f[:], in_=iop4[:])
        ge2 = im.tile([2 * MT, 1], f32, tag="ge2")
        nc.vector.tensor_scalar(out=ge2[:], in0=iop4f[:], scalar1=float(MT),
                                scalar2=None, op0=AOP.is_ge)
        idxa = im.tile([2 * MT, 1], f32, tag="idxa")
        nc.vector.scalar_tensor_tensor(out=idxa[:], in0=ge2[:],
                                       scalar=float(TT - MT), in1=iop4f[:],
                                       op0=AOP.mult, op1=AOP.add)
        idx4f = im.tile([2 * MT, 1], f32, tag="idx4f")
        nc.vector.scalar_tensor_tensor(out=idx4f[:], in0=cidb4[:],
                                       scalar=float(MT), in1=idxa[:],
                                       op0=AOP.mult, op1=AOP.add)
        idx4i = im.tile([2 * MT, 1], i32, tag="idx4i")
        nc.vector.tensor_copy(out=idx4i[:], in_=idx4f[:])
        pos4 = im.tile([2 * MT, 128], f32, tag="pos4")
        nc.gpsimd.indirect_dma_start(
            out=pos4[:], out_offset=None,
            in_=posd[0:1, :].rearrange("a (r p) -> (a r) p", p=128),
            in_offset=bass.IndirectOffsetOnAxis(ap=idx4i[:, :1], axis=0))
        nc.gpsimd.dma_start(out=posq[:, :], in_=pos4[:])
        nc.gpsimd.dma_start(out=posti[:],
                            in_=posq[:, :].rearrange("r p -> p r"))
        keep = im.tile([128, 2 * MT], f32, tag="keep")
        nc.vector.tensor_scalar(out=keep[:], in0=posti[:], scalar1=float(CAP),
                                scalar2=None, op0=AOP.is_lt)
        for m in range(MT):
            sa = im.tile([128, 1], f32, tag="sa")
            nc.vector.tensor_tensor(out=sa[:], in0=prb[m][:, 0:1],
                                    in1=keep[:, m:m + 1], op=AOP.mult)
            sb = im.tile([128, 1], f32, tag="sb")
            nc.vector.tensor_tensor(out=sb[:], in0=prb[m][:, 1:2],
                                    in1=keep[:, MT + m:MT + m + 1],
                                    op=AOP.mult)
            nc.vector.tensor_tensor(out=sm[m][:], in0=sa[:], in1=sb[:],
                                    op=AOP.add)
        nc.gpsimd.dma_start(
            out=b2b[:], in_=b2r[0:1, :].partition_broadcast(128).opt())
        for m in range(MT):
            nc.vector.tensor_scalar_mul(b2s[m][:], b2b[:], sm[m][:, 0:1])

    scan_cm.__exit__(None, None, None)

    # ============ F2: y = g @ w2.T, scaled epilogue ============
    with (tc.tile_pool(name="f2ps", bufs=1, space="PSUM") as f2ps,
          tc.tile_pool(name="f2o", bufs=4) as f2o):
        psq = [[f2ps.tile([128, 512], f32, tag=f"f2ps_{m}_{hq}",
                          name=f"f2ps_{m}_{hq}")
                for hq in range(4)] for m in range(MT)]
        for fc in range(FT):
            if fc + 4 < FT:
                load_w2(fc + 4)
            for m in range(MT):
                for hq in range(4):
                    nc.tensor.matmul(
                        out=psq[m][hq][:],
                        lhsT=g[fc][:, m * 128:(m + 1) * 128],
                        rhs=w2t[fc][:, hq * 512:(hq + 1) * 512],
                        start=(fc == 0), stop=(fc == FT - 1))
        for m in range(MT):
            for hq in range(4):
                o_t = f2o.tile([128, 512], f32, tag="o_t")
                nc.vector.scalar_tensor_tensor(
                    out=o_t[:], in0=psq[m][hq][:], scalar=sm[m][:, 0:1],
                    in1=b2s[m][:, hq * 512:(hq + 1) * 512],
                    op0=AOP.mult, op1=AOP.add)
                eng = nc.sync if (hq % 2 == 0) else nc.scalar
                eng.dma_start(
                    out=out[m * 128:(m + 1) * 128, hq * 512:(hq + 1) * 512],
                    in_=o_t[:])

    w2_cm.__exit__(None, None, None)
    g_cm.__exit__(None, None, None)


# ======================== host-side glue ========================

_CACHE = {}


def _prep_inputs(hidden_states, router_w, w1, b1, w2, b2):
    x = np.asarray(hidden_states, np.float32).reshape(-1, H)
    xT = np.ascontiguousarray(x.T)                       # [H, N] fp32
    w1Tm = np.asarray(w1, np.float32).T.astype(ml_dtypes.bfloat16)
    w2Tm = np.asarray(w2, np.float32).T.astype(ml_dtypes.bfloat16)
    w1tt = np.ascontiguousarray(
        w1Tm.reshape(HT, 128, FT, 128).transpose(2, 1, 0, 3)).reshape(
            FT, 128, H)
    base = {
        "w1tt": w1tt,
        "rwT": np.ascontiguousarray(np.asarray(router_w, np.float32).T),
        "w2T": np.ascontiguousarray(w2Tm),
        "b1t": np.ascontiguousarray(np.asarray(b1, np.float32).reshape(FT, 128).T),
        "b2r": np.asarray(b2, np.float32).reshape(1, H),
    }
    # router layout: [t_outer, h_inner, h_outer*t_inner] fp32
    xtmf = np.ascontiguousarray(
        xT.reshape(HT, 128, TT, 128).transpose(2, 1, 0, 3)).reshape(TT, 128, H)
    # FFN rhs layout: xT in [h_outer, h_inner, token] bf16, per-core slice
    xTb = xT.astype(ml_dtypes.bfloat16)
    ins = []
    for c in range(NCORES):
        m = dict(base)
        m["xtm"] = np.ascontiguousarray(xtmf[MT * c:MT * (c + 1)])
        m["xtc"] = np.ascontiguousarray(
            xTb[:, c * TOKC:(c + 1) * TOKC].reshape(HT, 128, TOKC)
            .transpose(1, 0, 2).reshape(128, HT * TOKC))
        m["cid"] = np.full((1, 1), float(c), np.float32)
        ins.append(m)
    return ins


def _get_nc():
    if "nc" not in _CACHE:
        apply()  # tile drain patch
        nc = bass.Bass(num_devices=NCORES)
        build_moe(nc)
        split_multi_waits(nc)
        _CACHE["nc"] = nc
    return _CACHE["nc"]


def kernel(hidden_states, router_w, w1, b1, w2, b2):
    from concourse.bass_utils import run_bass_kernel_spmd

    orig_shape = np.asarray(hidden_states).shape
    nc = _get_nc()
    ins = _prep_inputs(hidden_states, router_w, w1, b1, w2, b2)
    res = run_bass_kernel_spmd(nc, ins, core_ids=list(range(NCORES)))
    full = np.concatenate([res.results[c]["out"] for c in range(NCORES)], axis=0)
    return full.reshape(orig_shape).astype(np.float32)


# revision 26
# speedup vs baseline: 1.0171x; 1.0171x over previous
"""Trainium2 8-core MoE layer kernel (collapsed shared-expert FFN, Bass/Tile).

The reference MoE applies the SAME w1/b1/w2/b2 to every expert's slice of
the dispatch buffer, so the whole layer collapses algebraically to

    out[t] = s(t) * (gelu(x[t] @ w1.T + b1) @ w2.T + b2)
    s(t)   = p0(t)*keep0(t) + p1(t)*keep1(t)

where keep_k(t) is the capacity-survival flag from the slot-major
cumulative-count over the global top-2 assignment sequence.  Only s(t)
needs global routing info; the FFN itself is a dense per-token FFN.

Sharding: token-parallel, 256 tokens per core.  Each core runs the
fp32 router on its own tokens, AllGathers the 4KB of routing decisions,
recomputes the global cumsum locally (replicated, deterministic), and
runs the dense FFN on its 256 tokens with w1/w2 streamed from HBM in
bf16.  The routing/scan chain runs on vector/scalar/gpsimd engines
concurrently with the F1 matmul stream; s(t) is only consumed by the
final epilogue scaling.
"""
import numpy as np
import ml_dtypes

import concourse.bass as bass
import concourse.mybir as mybir
import concourse.tile as tile
import concourse.bass_isa as bass_isa

_PATCH_DOC = """Patch TileContext._drain_and_barrier: the stock version stuffs every
outstanding semaphore wait onto one SP Drain instruction; the installed
walrus rejects >1 sync wait per non-EventSemaphore instruction
("Too many sync wait commands"). Split the waits across a chain of SP
nops, then drain/barrier as before."""
import concourse.tile as tile_mod
from concourse.vector_clock import ScopedClock


def _patched_drain_and_barrier(self, tick_clock, wait_clock):
    nc = self.nc
    carrier = nc.sync.nop(nofuse=True, hint="drain_wait_carrier")
    wait_clock.add_sem_waits(
        carrier.ins, ScopedClock({None: tick_clock.global_clock})
    )
    waits = list(carrier.ins.sync_info.on_wait)
    if len(waits) > 1:
        carrier.ins.sync_info.on_wait = waits[:1]
        import bass_rust as _br
        for w in waits[1:]:
            extra = nc.sync.nop(nofuse=True, hint="drain_wait_carrier")
            extra.ins.sync_info = _br.SyncInfo(on_wait=[w], on_update=[])

    nc.sync.drain()
    nc.all_engine_barrier()
    assert self.sems is not None
    popped = nc._tile_sem_poison_stack.pop()
    assert popped is self._sem_poison
    nc.clear_and_free_semaphores(list(self.sems.allocated().values()))
    nc.all_engine_barrier()


def apply():
    tile_mod.TileContext._drain_and_barrier = _patched_drain_and_barrier


import concourse.mybir as mybir
import bass_rust as _br


def split_multi_waits(nc):
    """Walrus in this container accepts at most ONE sync wait per
    instruction. Hoist extra waits onto same-engine NoOps inserted
    immediately before the offending instruction."""
    ctr = 0
    for f in nc.m.functions:
        for b in f.blocks:
            insts = b.instructions
            need = any(
                inst.sync_info is not None and len(inst.sync_info.on_wait) > 1
                for inst in insts
            )
            if not need:
                continue
            out = []
            for inst in insts:
                si = inst.sync_info
                if si is not None and len(si.on_wait) > 1:
                    waits = list(si.on_wait)
                    for w in waits[:-1]:
                        nop = mybir.InstNoOp(name=f"I-wsplit-{ctr}", ins=[], outs=[])
                        ctr += 1
                        nop.engine = inst.engine
                        nop.sync_info = _br.SyncInfo(on_wait=[w], on_update=[])
                        out.append(nop)
                    si.on_wait = waits[-1:]
                out.append(inst)
            b.instructions = out
    return ctr


E, TOPK, CAP, H, F, N, NCORES = 8, 2, 512, 2048, 8192, 2048, 8
TT = N // 128                # 16 token tiles (global)
HT = H // 128                # 16 hidden tiles
FT = F // 128                # 64 ffn tiles
TOKC = N // NCORES           # 256 tokens per core
MT = TOKC // 128             # 2 local token tiles

f32 = mybir.dt.float32
f16 = mybir.dt.float16
bf16 = mybir.dt.bfloat16
i32 = mybir.dt.int32
AOP = mybir.AluOpType
AFT = mybir.ActivationFunctionType
AX = mybir.AxisListType


def build_moe(nc: bass.Bass):
    xtm = nc.dram_tensor("xtm", [MT, 128, H], f32, kind="ExternalInput")
    xtc = nc.dram_tensor("xtc", [128, HT * TOKC], bf16, kind="ExternalInput")
    rwT = nc.dram_tensor("rwT", [H, E], f32, kind="ExternalInput")
    w1T = nc.dram_tensor("w1tt", [FT, 128, HT * 128], bf16, kind="ExternalInput")
    w2T = nc.dram_tensor("w2T", [F, H], bf16, kind="ExternalInput")
    b1t = nc.dram_tensor("b1t", [128, FT], f32, kind="ExternalInput")
    b2r = nc.dram_tensor("b2r", [1, H], f32, kind="ExternalInput")
    cid = nc.dram_tensor("cid", [1, 1], f32, kind="ExternalInput")
    out = nc.dram_tensor("out", [TOKC, H], f32, kind="ExternalOutput")

    rloc = nc.dram_tensor("rloc", [2, TOKC], f32)
    rall = nc.dram_tensor("rall", [2 * NCORES, TOKC], f32, addr_space="Shared")
    posd = nc.dram_tensor("posd", [1, 2 * N], f32)
    posq = nc.dram_tensor("posq", [2 * MT, 128], f32)

    with tile.TileContext(nc, num_cores=NCORES) as tc:
        with tc.tile_pool(name="persist", bufs=1) as persist:
            _body(nc, tc, persist, xtm, xtc, rwT, w1T, w2T, b1t, b2r, cid,
                  out, rloc, rall, posd, posq)
    return nc


def _body(nc, tc, persist, xtm, xtc, rwT, w1T, w2T, b1t, b2r, cid, out,
          rloc, rall, posd, posq):
    # ---- persistent tiles ----
    xts = persist.tile([128, HT * TOKC], bf16, tag="xts")      # xT own tokens
    b2b = persist.tile([128, H], f32, tag="b2b")
    b2s = [persist.tile([128, H], f32, tag=f"b2s_{m}", name=f"b2s_{m}")
           for m in range(MT)]
    cidb4 = persist.tile([2 * MT, 1], f32, tag="cidb4")
    b1sb = persist.tile([128, FT], f32, tag="b1sb")
    rws = persist.tile([128, HT * E], f32, tag="rws")
    sm = [persist.tile([128, 1], f32, tag=f"sm_{m}", name=f"sm_{m}")
          for m in range(MT)]
    prb = [persist.tile([128, 2], f32, tag=f"prb_{m}", name=f"prb_{m}")
           for m in range(MT)]
    lsb2 = [persist.tile([128, E], f32, tag=f"lsb2_{m}", name=f"lsb2_{m}")
            for m in range(MT)]
    nm2 = [persist.tile([128, 1], f32, tag=f"nm2_{m}", name=f"nm2_{m}")
           for m in range(MT)]
    posti = persist.tile([128, 2 * MT], f32, tag="posti")

    # pools in LIFO lifetime order (longest-lived entered first)
    g_cm = tc.tile_pool(name="g", bufs=1)
    g_pool = g_cm.__enter__()
    w2_cm = tc.tile_pool(name="w2s", bufs=6)
    w2_pool = w2_cm.__enter__()
    scan_cm = tc.tile_pool(name="scan", bufs=1)
    scan_pool = scan_cm.__enter__()
    w1_cm = tc.tile_pool(name="w1s", bufs=10)
    w1_pool = w1_cm.__enter__()
    f1ps_cm = tc.tile_pool(name="f1ps", bufs=6, space="PSUM")
    f1ps_pool = f1ps_cm.__enter__()

    # initial small loads: gpsimd handles router-critical, scalar the rest
    nc.gpsimd.dma_start(out=cidb4[:],
                        in_=cid[0:1, :].partition_broadcast(2 * MT).opt())
    nc.gpsimd.dma_start(out=rws[:].rearrange("p (c e) -> p c e", e=E),
                        in_=rwT[:, :].rearrange("(c p) e -> p c e", p=128))
    nc.sync.dma_start(out=b1sb[:], in_=b1t[:, :])
    nc.scalar.dma_start(out=xts[:], in_=xtc[:, :])

    w1t = {}

    def load_w1(ft):
        t = w1_pool.tile([128, HT * 128], bf16, tag="w1t")
        w1t[ft] = t
        eng = nc.sync if (ft % 2 == 0) else nc.scalar
        eng.dma_start(out=t[:], in_=w1T[ft, :, :])

    for ft in range(8):
        load_w1(ft)

    g = [g_pool.tile([128, TOKC], bf16, tag=f"g_{ft}", name=f"g_{ft}")
         for ft in range(FT)]

    def f1_block(ft):
        if ft + 8 < FT:
            load_w1(ft + 8)
        ps = f1ps_pool.tile([128, TOKC], f32, tag="f1ps")
        for hc in range(HT):
            nc.tensor.matmul(
                out=ps[:], lhsT=w1t[ft][:, hc * 128:(hc + 1) * 128],
                rhs=xts[:, hc * TOKC:(hc + 1) * TOKC],
                start=(hc == 0), stop=(hc == HT - 1))
        nc.scalar.activation(out=g[ft][:], in_=ps[:], func=AFT.Gelu,
                             bias=b1sb[:, ft:ft + 1], scale=1.0)

    # F1 head: two tiles before the router matmuls so the PE queue never
    # waits on the (slightly later) xtm DMA.
    f1_block(0)
    f1_block(1)

    # ============ Router (own 256 tokens, fp32) + AllGather ============
    # Only the top-2 INDICES gate the AllGather (softmax is monotonic);
    # probabilities are computed after F1, overlapped with F2.
    with (tc.tile_pool(name="r_x", bufs=2) as r_x,
          tc.tile_pool(name="r_ps", bufs=2, space="PSUM") as r_ps,
          tc.tile_pool(name="r_sb", bufs=2) as r_sb):
        xtt = []
        for tt2 in range(MT):
            xt_t = r_x.tile([128, H], f32, tag="xt_t", name=f"xt_t{tt2}")
            nc.gpsimd.dma_start(out=xt_t[:], in_=xtm[tt2, :, :])
            xtt.append(xt_t)
        for tt2 in range(MT):
            ps = r_ps.tile([128, E], f32, tag="r_ps")
            for hc in range(HT):
                nc.tensor.matmul(
                    out=ps[:], lhsT=xtt[tt2][:, hc * 128:(hc + 1) * 128],
                    rhs=rws[:, hc * E:(hc + 1) * E],
                    start=(hc == 0), stop=(hc == HT - 1))
            nc.vector.tensor_copy(out=lsb2[tt2][:], in_=ps[:])
            mx = r_sb.tile([128, 1], f32, tag="mx")
            nc.vector.tensor_reduce(out=mx[:], in_=lsb2[tt2][:], op=AOP.max,
                                    axis=AX.X)
            nc.vector.tensor_scalar_mul(nm2[tt2][:], mx[:], -1.0)
            mx8 = r_sb.tile([128, 8], f32, tag="mx8")
            ix8 = r_sb.tile([128, 8], mybir.dt.uint32, tag="ix8")
            nc.vector.max_with_indices(out_max=mx8[:], out_indices=ix8[:],
                                       in_=lsb2[tt2][:])
            ev = r_sb.tile([128, 2], f32, tag="ev")
            nc.vector.tensor_copy(out=ev[:, 0:1], in_=ix8[:, 0:1])
            nc.vector.tensor_copy(out=ev[:, 1:2], in_=ix8[:, 1:2])
            nc.gpsimd.dma_start(
                out=rloc[:, tt2 * 128:(tt2 + 1) * 128].rearrange("a t -> t a"),
                in_=ev[:])
        nc.gpsimd.collective_compute(
            "AllGather", AOP.bypass,
            replica_groups=[list(range(NCORES))],
            ins=[rloc[:, :].opt()],
            outs=[rall[:, :].opt()])

    # w2 prefetch for the F1->F2 boundary (gpsimd queue, right after AG)
    w2t = {}

    def load_w2(fc, eng=None):
        t = w2_pool.tile([128, H], bf16, tag="w2t")
        w2t[fc] = t
        if eng is None:
            eng = nc.sync if (fc % 2 == 0) else nc.scalar
        eng.dma_start(out=t[:], in_=w2T[fc * 128:(fc + 1) * 128, :])

    for fc in range(4):
        load_w2(fc, eng=nc.gpsimd)

    # ============ Scan: global slot-major position per assignment ======
    # (vector engine; runs concurrently with F1 matmuls)
    iop8 = scan_pool.tile([E, 1], i32, tag="iop8")
    iop8f = scan_pool.tile([E, 1], f32, tag="iop8f")
    nc.gpsimd.iota(iop8[:], pattern=[[0, 1]], base=0, channel_multiplier=1)
    nc.vector.tensor_copy(out=iop8f[:], in_=iop8[:])
    e0b = scan_pool.tile([E, N], f16, tag="e0b")
    e1b = scan_pool.tile([E, N], f16, tag="e1b")
    rallv = rall[:, :].rearrange("(c a) t -> a c t", a=2)
    nc.gpsimd.dma_start(
        out=e0b[:].rearrange("p (c t) -> p c t", t=TOKC),
        in_=rallv[0:1, :, :].partition_broadcast(E).opt())
    nc.gpsimd.dma_start(
        out=e1b[:].rearrange("p (c t) -> p c t", t=TOKC),
        in_=rallv[1:2, :, :].partition_broadcast(E).opt())
    ohcat = scan_pool.tile([E, 2 * N], f16, tag="ohcat")
    nc.vector.tensor_scalar(out=ohcat[:, :N], in0=e0b[:], scalar1=iop8f[:],
                            scalar2=None, op0=AOP.is_equal)
    nc.vector.tensor_scalar(out=ohcat[:, N:], in0=e1b[:], scalar1=iop8f[:],
                            scalar2=None, op0=AOP.is_equal)
    ones2n = scan_pool.tile([E, 2 * N], f16, tag="ones2n")
    nc.vector.memset(ones2n[:], 1.0)
    cum = scan_pool.tile([E, 2 * N], f16, tag="cum")
    nc.vector.tensor_tensor_scan(out=cum[:], data0=ones2n[:], data1=ohcat[:],
                                 initial=0.0, op0=AOP.mult, op1=AOP.add)
    ohcum = scan_pool.tile([E, 2 * N], f16, tag="ohcum")
    nc.vector.tensor_tensor(out=ohcum[:], in0=ohcat[:], in1=cum[:],
                            op=AOP.mult)
    ones8 = scan_pool.tile([E, 1], f16, tag="ones8")
    nc.vector.memset(ones8[:], 1.0)
    posrow = scan_pool.tile([1, 2 * N], f32, tag="posrow")

    # ============ F1 body (tiles 2..63) ============
    for ft in range(2, FT):
        f1_block(ft)

    f1ps_cm.__exit__(None, None, None)
    w1_cm.__exit__(None, None, None)

    # pos extraction matmuls after the F1 stream (the AllGather + scan
    # complete well before F1 ends, so these never stall the PE).
    with tc.tile_pool(name="csps", bufs=4, space="PSUM") as csps:
        for ch in range(2 * N // 512):
            pps = csps.tile([1, 512], f32, tag="pps")
            nc.tensor.matmul(out=pps[:], lhsT=ones8[:],
                             rhs=ohcum[:, ch * 512:(ch + 1) * 512],
                             start=True, stop=True)
            nc.vector.tensor_scalar_add(
                posrow[:, ch * 512:(ch + 1) * 512], pps[:], -1.0)
    nc.gpsimd.dma_start(out=posd[:, :], in_=posrow[:])

    # ============ own-token keep/s: gather pos rows {2c+m, 16+2c+m} ======
    with tc.tile_pool(name="imath", bufs=1) as im:
        # late softmax: probabilities for own tokens (overlaps F2)
        for m in range(MT):
            ex = im.tile([128, E], f32, tag="ex", name=f"ex_{m}")
            ssum = im.tile([128, 1], f32, tag="ssum", name=f"ssum_{m}")
            nc.scalar.activation(out=ex[:], in_=lsb2[m][:], func=AFT.Exp,
                                 bias=nm2[m][:], scale=1.0, accum_out=ssum[:])
            rcp = im.tile([128, 1], f32, tag="rcp", name=f"rcp_{m}")
            nc.vector.reciprocal(out=rcp[:], in_=ssum[:])
            pr = im.tile([128, E], f32, tag="pr", name=f"pr_{m}")
            nc.vector.tensor_scalar_mul(pr[:], ex[:], rcp[:])
            mx8 = im.tile([128, 8], f32, tag="mx8", name=f"mx8_{m}")
            ix8d = im.tile([128, 8], mybir.dt.uint32, tag="ix8d",
                           name=f"ix8d_{m}")
            nc.vector.max_with_indices(out_max=mx8[:], out_indices=ix8d[:],
                                       in_=pr[:])
            nc.vector.tensor_copy(out=prb[m][:], in_=mx8[:, 0:2])

        iop4 = im.tile([2 * MT, 1], i32, tag="iop4")
        iop4f = im.tile([2 * MT, 1], f32, tag="iop4f")
        nc.gpsimd.iota(iop4[:], pattern=[[0, 1]], base=0, channel_multiplier=1)
        nc.vector.tensor_copy(out=iop4f[:], in_=iop4[:])
        ge2 = im.tile([2 * MT, 1], f32, tag="ge2")
        nc.vector.tensor_scalar(out=ge2[:], in0=iop4f[:], scalar1=float(MT),
                                scalar2=None, op0=AOP.is_ge)
        idxa = im.tile([2 * MT, 1], f32, tag="idxa")
        nc.vector.scalar_tensor_tensor(out=idxa[:], in0=ge2[:],
                                       scalar=float(TT - MT), in1=iop4f[:],
                                       op0=AOP.mult, op1=AOP.add)
        idx4f = im.tile([2 * MT, 1], f32, tag="idx4f")
        nc.vector.scalar_tensor_tensor(out=idx4f[:], in0=cidb4[:],
                                       scalar=float(MT), in1=idxa[:],
                                       op0=AOP.mult, op1=AOP.add)
        idx4i = im.tile([2 * MT, 1], i32, tag="idx4i")
        nc.vector.tensor_copy(out=idx4i[:], in_=idx4f[:])
        pos4 = im.tile([2 * MT, 128], f32, tag="pos4")
        nc.gpsimd.indirect_dma_start(
            out=pos4[:], out_offset=None,
            in_=posd[0:1, :].rearrange("a (r p) -> (a r) p", p=128),
            in_offset=bass.IndirectOffsetOnAxis(ap=idx4i[:, :1], axis=0))
        nc.gpsimd.dma_start(out=posq[:, :], in_=pos4[:])
        nc.gpsimd.dma_start(out=posti[:],
                            in_=posq[:, :].rearrange("r p -> p r"))
        keep = im.tile([128, 2 * MT], f32, tag="keep")
        nc.vector.tensor_scalar(out=keep[:], in0=posti[:], scalar1=float(CAP),
                                scalar2=None, op0=AOP.is_lt)
        for m in range(MT):
            sa = im.tile([128, 1], f32, tag="sa")
            nc.vector.tensor_tensor(out=sa[:], in0=prb[m][:, 0:1],
                                    in1=keep[:, m:m + 1], op=AOP.mult)
            sb = im.tile([128, 1], f32, tag="sb")
            nc.vector.tensor_tensor(out=sb[:], in0=prb[m][:, 1:2],
                                    in1=keep[:, MT + m:MT + m + 1],
                                    op=AOP.mult)
            nc.vector.tensor_tensor(out=sm[m][:], in0=sa[:], in1=sb[:],
                                    op=AOP.add)
        nc.gpsimd.dma_start(
            out=b2b[:], in_=b2r[0:1, :].partition_broadcast(128).opt())
        for m in range(MT):
            nc.vector.tensor_scalar_mul(b2s[m][:], b2b[:], sm[m][:, 0:1])

    scan_cm.__exit__(None, None, None)

    # ============ F2: y = g @ w2.T, scaled epilogue ============
    with (tc.tile_pool(name="f2ps", bufs=1, space="PSUM") as f2ps,
          tc.tile_pool(name="f2o", bufs=4) as f2o):
        psq = [[f2ps.tile([128, 512], f32, tag=f"f2ps_{m}_{hq}",
                          name=f"f2ps_{m}_{hq}")
                for hq in range(4)] for m in range(MT)]
        for fc in range(FT):
            if fc + 4 < FT:
                load_w2(fc + 4)
            for m in range(MT):
                for hq in range(4):
                    nc.tensor.matmul(
                        out=psq[m][hq][:],
                        lhsT=g[fc][:, m * 128:(m + 1) * 128],
                        rhs=w2t[fc][:, hq * 512:(hq + 1) * 512],
                        start=(fc == 0), stop=(fc == FT - 1))
        for m in range(MT):
            for hq in range(4):
                o_t = f2o.tile([128, 512], f32, tag="o_t")
                nc.vector.scalar_tensor_tensor(
                    out=o_t[:], in0=psq[m][hq][:], scalar=sm[m][:, 0:1],
                    in1=b2s[m][:, hq * 512:(hq + 1) * 512],
                    op0=AOP.mult, op1=AOP.add)
                eng = nc.sync if (hq % 2 == 0) else nc.scalar
                eng.dma_start(
                    out=out[m * 128:(m + 1) * 128, hq * 512:(hq + 1) * 512],
                    in_=o_t[:])

    w2_cm.__exit__(None, None, None)
    g_cm.__exit__(None, None, None)


# ======================== host-side glue ========================

_CACHE = {}


def _prep_inputs(hidden_states, router_w, w1, b1, w2, b2):
    x = np.asarray(hidden_states, np.float32).reshape(-1, H)
    xT = np.ascontiguousarray(x.T)                       # [H, N] fp32
    w1Tm = np.asarray(w1, np.float32).T.astype(ml_dtypes.bfloat16)
    w2Tm = np.asarray(w2, np.float32).T.astype(ml_dtypes.bfloat16)
    w1tt = np.ascontiguousarray(
        w1Tm.reshape(HT, 128, FT, 128).transpose(2, 1, 0, 3)).reshape(
            FT, 128, H)
    base = {
        "w1tt": w1tt,
        "rwT": np.ascontiguousarray(np.asarray(router_w, np.float32).T),
        "w2T": np.ascontiguousarray(w2Tm),
        "b1t": np.ascontiguousarray(np.asarray(b1, np.float32).reshape(FT, 128).T),
        "b2r": np.asarray(b2, np.float32).reshape(1, H),
    }
    # router layout: [t_outer, h_inner, h_outer*t_inner] fp32
    xtmf = np.ascontiguousarray(
        xT.reshape(HT, 128, TT, 128).transpose(2, 1, 0, 3)).reshape(TT, 128, H)
    # FFN rhs layout: xT in [h_outer, h_inner, token] bf16, per-core slice
    xTb = xT.astype(ml_dtypes.bfloat16)
    ins = []
    for c in range(NCORES):
        m = dict(base)
        m["xtm"] = np.ascontiguousarray(xtmf[MT * c:MT * (c + 1)])
        m["xtc"] = np.ascontiguousarray(
            xTb[:, c * TOKC:(c + 1) * TOKC].reshape(HT, 128, TOKC)
            .transpose(1, 0, 2).reshape(128, HT * TOKC))
        m["cid"] = np.full((1, 1), float(c), np.float32)
        ins.append(m)
    return ins


def _get_nc():
    if "nc" not in _CACHE:
        apply()  # tile drain patch
        nc = bass.Bass(num_devices=NCORES)
        build_moe(nc)
        split_multi_waits(nc)
        _CACHE["nc"] = nc
    return _CACHE["nc"]


def kernel(hidden_states, router_w, w1, b1, w2, b2):
    from concourse.bass_utils import run_bass_kernel_spmd

    orig_shape = np.asarray(hidden_states).shape
    nc = _get_nc()
    ins = _prep_inputs(hidden_states, router_w, w1, b1, w2, b2)
    res = run_bass_kernel_spmd(nc, ins, core_ids=list(range(NCORES)))
    full = np.concatenate([res.results[c]["out"] for c in range(NCORES)], axis=0)
    return full.reshape(orig_shape).astype(np.float32)


# revision 27
# speedup vs baseline: 1.0348x; 1.0174x over previous
"""Trainium2 8-core MoE layer kernel (collapsed shared-expert FFN, Bass/Tile).

The reference MoE applies the SAME w1/b1/w2/b2 to every expert's slice of
the dispatch buffer, so the whole layer collapses algebraically to

    out[t] = s(t) * (gelu(x[t] @ w1.T + b1) @ w2.T + b2)
    s(t)   = p0(t)*keep0(t) + p1(t)*keep1(t)

where keep_k(t) is the capacity-survival flag from the slot-major
cumulative-count over the global top-2 assignment sequence.  Only s(t)
needs global routing info; the FFN itself is a dense per-token FFN.

Sharding: token-parallel, 256 tokens per core.  Each core runs the
fp32 router on its own tokens, AllGathers the 4KB of routing decisions,
recomputes the global cumsum locally (replicated, deterministic), and
runs the dense FFN on its 256 tokens with w1/w2 streamed from HBM in
bf16.  The routing/scan chain runs on vector/scalar/gpsimd engines
concurrently with the F1 matmul stream; s(t) is only consumed by the
final epilogue scaling.
"""
import numpy as np
import ml_dtypes

import concourse.bass as bass
import concourse.mybir as mybir
import concourse.tile as tile
import concourse.bass_isa as bass_isa

_PATCH_DOC = """Patch TileContext._drain_and_barrier: the stock version stuffs every
outstanding semaphore wait onto one SP Drain instruction; the installed
walrus rejects >1 sync wait per non-EventSemaphore instruction
("Too many sync wait commands"). Split the waits across a chain of SP
nops, then drain/barrier as before."""
import concourse.tile as tile_mod
from concourse.vector_clock import ScopedClock


def _patched_drain_and_barrier(self, tick_clock, wait_clock):
    nc = self.nc
    carrier = nc.sync.nop(nofuse=True, hint="drain_wait_carrier")
    wait_clock.add_sem_waits(
        carrier.ins, ScopedClock({None: tick_clock.global_clock})
    )
    waits = list(carrier.ins.sync_info.on_wait)
    if len(waits) > 1:
        carrier.ins.sync_info.on_wait = waits[:1]
        import bass_rust as _br
        for w in waits[1:]:
            extra = nc.sync.nop(nofuse=True, hint="drain_wait_carrier")
            extra.ins.sync_info = _br.SyncInfo(on_wait=[w], on_update=[])

    nc.sync.drain()
    nc.all_engine_barrier()
    assert self.sems is not None
    popped = nc._tile_sem_poison_stack.pop()
    assert popped is self._sem_poison
    nc.clear_and_free_semaphores(list(self.sems.allocated().values()))
    nc.all_engine_barrier()


def apply():
    tile_mod.TileContext._drain_and_barrier = _patched_drain_and_barrier


import concourse.mybir as mybir
import bass_rust as _br


def split_multi_waits(nc):
    """Walrus in this container accepts at most ONE sync wait per
    instruction. Hoist extra waits onto same-engine NoOps inserted
    immediately before the offending instruction."""
    ctr = 0
    for f in nc.m.functions:
        for b in f.blocks:
            insts = b.instructions
            need = any(
                inst.sync_info is not None and len(inst.sync_info.on_wait) > 1
                for inst in insts
            )
            if not need:
                continue
            out = []
            for inst in insts:
                si = inst.sync_info
                if si is not None and len(si.on_wait) > 1:
                    waits = list(si.on_wait)
                    for w in waits[:-1]:
                        nop = mybir.InstNoOp(name=f"I-wsplit-{ctr}", ins=[], outs=[])
                        ctr += 1
                        nop.engine = inst.engine
                        nop.sync_info = _br.SyncInfo(on_wait=[w], on_update=[])
                        out.append(nop)
                    si.on_wait = waits[-1:]
                out.append(inst)
            b.instructions = out
    return ctr


E, TOPK, CAP, H, F, N, NCORES = 8, 2, 512, 2048, 8192, 2048, 8
TT = N // 128                # 16 token tiles (global)
HT = H // 128                # 16 hidden tiles
FT = F // 128                # 64 ffn tiles
TOKC = N // NCORES           # 256 tokens per core
MT = TOKC // 128             # 2 local token tiles

f32 = mybir.dt.float32
f16 = mybir.dt.float16
bf16 = mybir.dt.bfloat16
i32 = mybir.dt.int32
AOP = mybir.AluOpType
AFT = mybir.ActivationFunctionType
AX = mybir.AxisListType


def build_moe(nc: bass.Bass):
    xtm = nc.dram_tensor("xtm", [MT, 128, H], f32, kind="ExternalInput")
    xtc = nc.dram_tensor("xtc", [128, HT * TOKC], bf16, kind="ExternalInput")
    rwT = nc.dram_tensor("rwT", [128, HT * E], f32, kind="ExternalInput")
    w1T = nc.dram_tensor("w1tt", [FT, 128, HT * 128], bf16, kind="ExternalInput")
    w2T = nc.dram_tensor("w2T", [F, H], bf16, kind="ExternalInput")
    b1t = nc.dram_tensor("b1t", [128, FT], f32, kind="ExternalInput")
    b2r = nc.dram_tensor("b2r", [1, H], f32, kind="ExternalInput")
    cid = nc.dram_tensor("cid", [1, 1], f32, kind="ExternalInput")
    out = nc.dram_tensor("out", [TOKC, H], f32, kind="ExternalOutput")

    rloc = nc.dram_tensor("rloc", [2, TOKC], f32)
    rall = nc.dram_tensor("rall", [2 * NCORES, TOKC], f32, addr_space="Shared")
    posd = nc.dram_tensor("posd", [1, 2 * N], f32)
    posq = nc.dram_tensor("posq", [2 * MT, 128], f32)

    with tile.TileContext(nc, num_cores=NCORES) as tc:
        with tc.tile_pool(name="persist", bufs=1) as persist:
            _body(nc, tc, persist, xtm, xtc, rwT, w1T, w2T, b1t, b2r, cid,
                  out, rloc, rall, posd, posq)
    return nc


def _body(nc, tc, persist, xtm, xtc, rwT, w1T, w2T, b1t, b2r, cid, out,
          rloc, rall, posd, posq):
    # ---- persistent tiles ----
    xts = persist.tile([128, HT * TOKC], bf16, tag="xts")      # xT own tokens
    b2b = persist.tile([128, H], f32, tag="b2b")
    b2s = [persist.tile([128, H], f32, tag=f"b2s_{m}", name=f"b2s_{m}")
           for m in range(MT)]
    cidb4 = persist.tile([2 * MT, 1], f32, tag="cidb4")
    b1sb = persist.tile([128, FT], f32, tag="b1sb")
    rws = persist.tile([128, HT * E], f32, tag="rws")
    sm = [persist.tile([128, 1], f32, tag=f"sm_{m}", name=f"sm_{m}")
          for m in range(MT)]
    prb = [persist.tile([128, 2], f32, tag=f"prb_{m}", name=f"prb_{m}")
           for m in range(MT)]
    lsb2 = [persist.tile([128, E], f32, tag=f"lsb2_{m}", name=f"lsb2_{m}")
            for m in range(MT)]
    nm2 = [persist.tile([128, 1], f32, tag=f"nm2_{m}", name=f"nm2_{m}")
           for m in range(MT)]
    posti = persist.tile([128, 2 * MT], f32, tag="posti")

    # pools in LIFO lifetime order (longest-lived entered first)
    g_cm = tc.tile_pool(name="g", bufs=1)
    g_pool = g_cm.__enter__()
    w2_cm = tc.tile_pool(name="w2s", bufs=6)
    w2_pool = w2_cm.__enter__()
    scan_cm = tc.tile_pool(name="scan", bufs=1)
    scan_pool = scan_cm.__enter__()
    w1_cm = tc.tile_pool(name="w1s", bufs=10)
    w1_pool = w1_cm.__enter__()
    f1ps_cm = tc.tile_pool(name="f1ps", bufs=6, space="PSUM")
    f1ps_pool = f1ps_cm.__enter__()

    # initial small loads: gpsimd handles router-critical, scalar the rest
    nc.gpsimd.dma_start(out=cidb4[:],
                        in_=cid[0:1, :].partition_broadcast(2 * MT).opt())
    nc.gpsimd.dma_start(out=rws[:], in_=rwT[:, :])
    nc.sync.dma_start(out=b1sb[:], in_=b1t[:, :])
    nc.scalar.dma_start(out=xts[:], in_=xtc[:, :])

    w1t = {}

    def load_w1(ft):
        t = w1_pool.tile([128, HT * 128], bf16, tag="w1t")
        w1t[ft] = t
        eng = nc.sync if (ft % 2 == 0) else nc.scalar
        eng.dma_start(out=t[:], in_=w1T[ft, :, :])

    for ft in range(8):
        load_w1(ft)

    g = [g_pool.tile([128, TOKC], bf16, tag=f"g_{ft}", name=f"g_{ft}")
         for ft in range(FT)]

    def f1_block(ft):
        if ft + 8 < FT:
            load_w1(ft + 8)
        ps = f1ps_pool.tile([128, TOKC], f32, tag="f1ps")
        for hc in range(HT):
            nc.tensor.matmul(
                out=ps[:], lhsT=w1t[ft][:, hc * 128:(hc + 1) * 128],
                rhs=xts[:, hc * TOKC:(hc + 1) * TOKC],
                start=(hc == 0), stop=(hc == HT - 1))
        nc.scalar.activation(out=g[ft][:], in_=ps[:], func=AFT.Gelu,
                             bias=b1sb[:, ft:ft + 1], scale=1.0)

    # F1 head: six tiles before the router matmuls so the PE stream runs
    # dense from the start (HAM warm-up) while xtm/rws land.
    for ft in range(6):
        f1_block(ft)

    # ============ Router (own 256 tokens, fp32) + AllGather ============
    # Only the top-2 INDICES gate the AllGather (softmax is monotonic);
    # probabilities are computed after F1, overlapped with F2.
    with (tc.tile_pool(name="r_x", bufs=2) as r_x,
          tc.tile_pool(name="r_ps", bufs=2, space="PSUM") as r_ps,
          tc.tile_pool(name="r_sb", bufs=2) as r_sb):
        xtt = []
        for tt2 in range(MT):
            xt_t = r_x.tile([128, H], f32, tag="xt_t", name=f"xt_t{tt2}")
            nc.gpsimd.dma_start(out=xt_t[:], in_=xtm[tt2, :, :])
            xtt.append(xt_t)
        for tt2 in range(MT):
            ps = r_ps.tile([128, E], f32, tag="r_ps")
            for hc in range(HT):
                nc.tensor.matmul(
                    out=ps[:], lhsT=xtt[tt2][:, hc * 128:(hc + 1) * 128],
                    rhs=rws[:, hc * E:(hc + 1) * E],
                    start=(hc == 0), stop=(hc == HT - 1))
            nc.vector.tensor_copy(out=lsb2[tt2][:], in_=ps[:])
            mx = r_sb.tile([128, 1], f32, tag="mx")
            nc.vector.tensor_reduce(out=mx[:], in_=lsb2[tt2][:], op=AOP.max,
                                    axis=AX.X)
            nc.vector.tensor_scalar_mul(nm2[tt2][:], mx[:], -1.0)
            mx8 = r_sb.tile([128, 8], f32, tag="mx8")
            ix8 = r_sb.tile([128, 8], mybir.dt.uint32, tag="ix8")
            nc.vector.max_with_indices(out_max=mx8[:], out_indices=ix8[:],
                                       in_=lsb2[tt2][:])
            ev = r_sb.tile([128, 2], f32, tag="ev")
            nc.vector.tensor_copy(out=ev[:, 0:1], in_=ix8[:, 0:1])
            nc.vector.tensor_copy(out=ev[:, 1:2], in_=ix8[:, 1:2])
            nc.gpsimd.dma_start(
                out=rloc[:, tt2 * 128:(tt2 + 1) * 128].rearrange("a t -> t a"),
                in_=ev[:])
        nc.gpsimd.collective_compute(
            "AllGather", AOP.bypass,
            replica_groups=[list(range(NCORES))],
            ins=[rloc[:, :].opt()],
            outs=[rall[:, :].opt()])

    # w2 prefetch for the F1->F2 boundary (gpsimd queue, right after AG)
    w2t = {}

    def load_w2(fc, eng=None):
        t = w2_pool.tile([128, H], bf16, tag="w2t")
        w2t[fc] = t
        if eng is None:
            eng = nc.sync if (fc % 2 == 0) else nc.scalar
        eng.dma_start(out=t[:], in_=w2T[fc * 128:(fc + 1) * 128, :])

    for fc in range(4):
        load_w2(fc, eng=nc.gpsimd)

    # ============ Scan: global slot-major position per assignment ======
    # (vector engine; runs concurrently with F1 matmuls)
    iop8 = scan_pool.tile([E, 1], i32, tag="iop8")
    iop8f = scan_pool.tile([E, 1], f32, tag="iop8f")
    nc.gpsimd.iota(iop8[:], pattern=[[0, 1]], base=0, channel_multiplier=1)
    nc.vector.tensor_copy(out=iop8f[:], in_=iop8[:])
    e0b = scan_pool.tile([E, N], f16, tag="e0b")
    e1b = scan_pool.tile([E, N], f16, tag="e1b")
    rallv = rall[:, :].rearrange("(c a) t -> a c t", a=2)
    nc.gpsimd.dma_start(
        out=e0b[:].rearrange("p (c t) -> p c t", t=TOKC),
        in_=rallv[0:1, :, :].partition_broadcast(E).opt())
    nc.gpsimd.dma_start(
        out=e1b[:].rearrange("p (c t) -> p c t", t=TOKC),
        in_=rallv[1:2, :, :].partition_broadcast(E).opt())
    ohcat = scan_pool.tile([E, 2 * N], f16, tag="ohcat")
    nc.vector.tensor_scalar(out=ohcat[:, :N], in0=e0b[:], scalar1=iop8f[:],
                            scalar2=None, op0=AOP.is_equal)
    nc.vector.tensor_scalar(out=ohcat[:, N:], in0=e1b[:], scalar1=iop8f[:],
                            scalar2=None, op0=AOP.is_equal)
    ones2n = scan_pool.tile([E, 2 * N], f16, tag="ones2n")
    nc.vector.memset(ones2n[:], 1.0)
    cum = scan_pool.tile([E, 2 * N], f16, tag="cum")
    nc.vector.tensor_tensor_scan(out=cum[:], data0=ones2n[:], data1=ohcat[:],
                                 initial=0.0, op0=AOP.mult, op1=AOP.add)
    ohcum = scan_pool.tile([E, 2 * N], f16, tag="ohcum")
    nc.vector.tensor_tensor(out=ohcum[:], in0=ohcat[:], in1=cum[:],
                            op=AOP.mult)
    ones8 = scan_pool.tile([E, 1], f16, tag="ones8")
    nc.vector.memset(ones8[:], 1.0)
    posrow = scan_pool.tile([1, 2 * N], f32, tag="posrow")

    # ============ F1 body (tiles 6..63) ============
    for ft in range(6, FT):
        f1_block(ft)

    f1ps_cm.__exit__(None, None, None)
    w1_cm.__exit__(None, None, None)

    # pos extraction matmuls after the F1 stream (the AllGather + scan
    # complete well before F1 ends, so these never stall the PE).
    with tc.tile_pool(name="csps", bufs=4, space="PSUM") as csps:
        for ch in range(2 * N // 512):
            pps = csps.tile([1, 512], f32, tag="pps")
            nc.tensor.matmul(out=pps[:], lhsT=ones8[:],
                             rhs=ohcum[:, ch * 512:(ch + 1) * 512],
                             start=True, stop=True)
            nc.vector.tensor_scalar_add(
                posrow[:, ch * 512:(ch + 1) * 512], pps[:], -1.0)
    nc.gpsimd.dma_start(out=posd[:, :], in_=posrow[:])

    # ============ own-token keep/s: gather pos rows {2c+m, 16+2c+m} ======
    with tc.tile_pool(name="imath", bufs=1) as im:
        # late softmax: probabilities for own tokens (overlaps F2)
        for m in range(MT):
            ex = im.tile([128, E], f32, tag="ex", name=f"ex_{m}")
            ssum = im.tile([128, 1], f32, tag="ssum", name=f"ssum_{m}")
            nc.scalar.activation(out=ex[:], in_=lsb2[m][:], func=AFT.Exp,
                                 bias=nm2[m][:], scale=1.0, accum_out=ssum[:])
            rcp = im.tile([128, 1], f32, tag="rcp", name=f"rcp_{m}")
            nc.vector.reciprocal(out=rcp[:], in_=ssum[:])
            pr = im.tile([128, E], f32, tag="pr", name=f"pr_{m}")
            nc.vector.tensor_scalar_mul(pr[:], ex[:], rcp[:])
            mx8 = im.tile([128, 8], f32, tag="mx8", name=f"mx8_{m}")
            ix8d = im.tile([128, 8], mybir.dt.uint32, tag="ix8d",
                           name=f"ix8d_{m}")
            nc.vector.max_with_indices(out_max=mx8[:], out_indices=ix8d[:],
                                       in_=pr[:])
            nc.vector.tensor_copy(out=prb[m][:], in_=mx8[:, 0:2])

        iop4 = im.tile([2 * MT, 1], i32, tag="iop4")
        iop4f = im.tile([2 * MT, 1], f32, tag="iop4f")
        nc.gpsimd.iota(iop4[:], pattern=[[0, 1]], base=0, channel_multiplier=1)
        nc.vector.tensor_copy(out=iop4f[:], in_=iop4[:])
        ge2 = im.tile([2 * MT, 1], f32, tag="ge2")
        nc.vector.tensor_scalar(out=ge2[:], in0=iop4f[:], scalar1=float(MT),
                                scalar2=None, op0=AOP.is_ge)
        idxa = im.tile([2 * MT, 1], f32, tag="idxa")
        nc.vector.scalar_tensor_tensor(out=idxa[:], in0=ge2[:],
                                       scalar=float(TT - MT), in1=iop4f[:],
                                       op0=AOP.mult, op1=AOP.add)
        idx4f = im.tile([2 * MT, 1], f32, tag="idx4f")
        nc.vector.scalar_tensor_tensor(out=idx4f[:], in0=cidb4[:],
                                       scalar=float(MT), in1=idxa[:],
                                       op0=AOP.mult, op1=AOP.add)
        idx4i = im.tile([2 * MT, 1], i32, tag="idx4i")
        nc.vector.tensor_copy(out=idx4i[:], in_=idx4f[:])
        pos4 = im.tile([2 * MT, 128], f32, tag="pos4")
        nc.gpsimd.indirect_dma_start(
            out=pos4[:], out_offset=None,
            in_=posd[0:1, :].rearrange("a (r p) -> (a r) p", p=128),
            in_offset=bass.IndirectOffsetOnAxis(ap=idx4i[:, :1], axis=0))
        nc.gpsimd.dma_start(out=posq[:, :], in_=pos4[:])
        nc.gpsimd.dma_start(out=posti[:],
                            in_=posq[:, :].rearrange("r p -> p r"))
        keep = im.tile([128, 2 * MT], f32, tag="keep")
        nc.vector.tensor_scalar(out=keep[:], in0=posti[:], scalar1=float(CAP),
                                scalar2=None, op0=AOP.is_lt)
        for m in range(MT):
            sa = im.tile([128, 1], f32, tag="sa")
            nc.vector.tensor_tensor(out=sa[:], in0=prb[m][:, 0:1],
                                    in1=keep[:, m:m + 1], op=AOP.mult)
            sb = im.tile([128, 1], f32, tag="sb")
            nc.vector.tensor_tensor(out=sb[:], in0=prb[m][:, 1:2],
                                    in1=keep[:, MT + m:MT + m + 1],
                                    op=AOP.mult)
            nc.vector.tensor_tensor(out=sm[m][:], in0=sa[:], in1=sb[:],
                                    op=AOP.add)
        nc.gpsimd.dma_start(
            out=b2b[:], in_=b2r[0:1, :].partition_broadcast(128).opt())
        for m in range(MT):
            nc.vector.tensor_scalar_mul(b2s[m][:], b2b[:], sm[m][:, 0:1])

    scan_cm.__exit__(None, None, None)

    # ============ F2: y = g @ w2.T, scaled epilogue ============
    with (tc.tile_pool(name="f2ps", bufs=1, space="PSUM") as f2ps,
          tc.tile_pool(name="f2o", bufs=4) as f2o):
        psq = [[f2ps.tile([128, 512], f32, tag=f"f2ps_{m}_{hq}",
                          name=f"f2ps_{m}_{hq}")
                for hq in range(4)] for m in range(MT)]
        for fc in range(FT):
            if fc + 4 < FT:
                load_w2(fc + 4)
            for m in range(MT):
                for hq in range(4):
                    nc.tensor.matmul(
                        out=psq[m][hq][:],
                        lhsT=g[fc][:, m * 128:(m + 1) * 128],
                        rhs=w2t[fc][:, hq * 512:(hq + 1) * 512],
                        start=(fc == 0), stop=(fc == FT - 1))
        for m in range(MT):
            for hq in range(4):
                o_t = f2o.tile([128, 512], f32, tag="o_t")
                nc.vector.scalar_tensor_tensor(
                    out=o_t[:], in0=psq[m][hq][:], scalar=sm[m][:, 0:1],
                    in1=b2s[m][:, hq * 512:(hq + 1) * 512],
                    op0=AOP.mult, op1=AOP.add)
                eng = nc.sync if (hq % 2 == 0) else nc.scalar
                eng.dma_start(
                    out=out[m * 128:(m + 1) * 128, hq * 512:(hq + 1) * 512],
                    in_=o_t[:])

    w2_cm.__exit__(None, None, None)
    g_cm.__exit__(None, None, None)


# ======================== host-side glue ========================

_CACHE = {}


def _prep_inputs(hidden_states, router_w, w1, b1, w2, b2):
    x = np.asarray(hidden_states, np.float32).reshape(-1, H)
    xT = np.ascontiguousarray(x.T)                       # [H, N] fp32
    w1Tm = np.asarray(w1, np.float32).T.astype(ml_dtypes.bfloat16)
    w2Tm = np.asarray(w2, np.float32).T.astype(ml_dtypes.bfloat16)
    w1tt = np.ascontiguousarray(
        w1Tm.reshape(HT, 128, FT, 128).transpose(2, 1, 0, 3)).reshape(
            FT, 128, H)
    base = {
        "w1tt": w1tt,
        "rwT": np.ascontiguousarray(np.asarray(router_w, np.float32).T
                            .reshape(HT, 128, E).transpose(1, 0, 2)
                            .reshape(128, HT * E)),
        "w2T": np.ascontiguousarray(w2Tm),
        "b1t": np.ascontiguousarray(np.asarray(b1, np.float32).reshape(FT, 128).T),
        "b2r": np.asarray(b2, np.float32).reshape(1, H),
    }
    # router layout: [t_outer, h_inner, h_outer*t_inner] fp32
    xtmf = np.ascontiguousarray(
        xT.reshape(HT, 128, TT, 128).transpose(2, 1, 0, 3)).reshape(TT, 128, H)
    # FFN rhs layout: xT in [h_outer, h_inner, token] bf16, per-core slice
    xTb = xT.astype(ml_dtypes.bfloat16)
    ins = []
    for c in range(NCORES):
        m = dict(base)
        m["xtm"] = np.ascontiguousarray(xtmf[MT * c:MT * (c + 1)])
        m["xtc"] = np.ascontiguousarray(
            xTb[:, c * TOKC:(c + 1) * TOKC].reshape(HT, 128, TOKC)
            .transpose(1, 0, 2).reshape(128, HT * TOKC))
        m["cid"] = np.full((1, 1), float(c), np.float32)
        ins.append(m)
    return ins


def _get_nc():
    if "nc" not in _CACHE:
        apply()  # tile drain patch
        nc = bass.Bass(num_devices=NCORES)
        build_moe(nc)
        split_multi_waits(nc)
        _CACHE["nc"] = nc
    return _CACHE["nc"]


def kernel(hidden_states, router_w, w1, b1, w2, b2):
    from concourse.bass_utils import run_bass_kernel_spmd

    orig_shape = np.asarray(hidden_states).shape
    nc = _get_nc()
    ins = _prep_inputs(hidden_states, router_w, w1, b1, w2, b2)
    res = run_bass_kernel_spmd(nc, ins, core_ids=list(range(NCORES)))
    full = np.concatenate([res.results[c]["out"] for c in range(NCORES)], axis=0)
    return full.reshape(orig_shape).astype(np.float32)
